# revision 19
# baseline (speedup 1.0000x reference)
"""Trainium2 Bass kernel for nn_Attention_48687749268214.

Self-attention with pair-bias: LN(x) -> qkv -> q/k LN -> heads,
bias = einsum('bijc,hc->bhij', LN(pair), w_bias), softmax(qk/8+bias) @ v -> proj.

Sharding: sequence-shard the i axis across 8 cores (64 query rows each).
Each core gets its pair slice pair[i0:i0+64] (j-rolled so that the core's own
query rows sit at local rows 0:63 in its rolled copy of x), computes its own
64 output rows with no collectives; host concatenates.

Pair-path math (v4): with pn_g folded into wg on host,
  bias[h, ij] = r_ij * (raw[h, ij] - mu_ij * cs[h]) + sb[h]
where raw = pair @ wg, cs = colsum(wg), and sb is constant along j so softmax
drops it (exact). The kernel never normalizes pair and computes NO stats on
the critical path:
  - mu: a 13th all-ones column in wg makes the bias matmul emit 768*mu as
    psum row 12, which rides into bias_r with the raw panels.
  - var: per-tile sum(pair^2) passes split across ACT/DVE/GpSimd engines,
    collected into a [128, 256] tile (col = 4*panel + jblock).
  - tail: cheap [64, 512] map ops build mu_map/rmap, and per-head
    bfix = (raw - cs_h*mu_map) * rmap feeds softmax.
So PE's pair work (transposes + matmuls) depends only on the pair DMA.
"""

import sys

sys.path.insert(0, "/opt/trn_rl_repo")

from contextlib import ExitStack

import ml_dtypes
import numpy as np

import concourse.bass as bass
import concourse.tile as tile
from concourse import bacc, mybir
from concourse.bass_utils import run_bass_kernel_spmd
from concourse.masks import make_identity

F32 = mybir.dt.float32
BF16 = mybir.dt.bfloat16
AF = mybir.ActivationFunctionType
OP = mybir.AluOpType

C = 768
H = 12
HD = 64
N = 512
NCORES = 8
IB = N // NCORES  # 64 i rows per core
NIJ = IB * N  # 32768 pair rows per core
EPS = 1e-5
RC = 1.0 / C

# engine assignment knobs for the per-panel sumsq passes (tile idx 0-3)
SQ_ACT = (0, 1)
SQ_DVE = (2, 3)
# psum->sbuf drain assignment for pair transposes (c-chunk 0-5)
DR_DVE = (1, 2, 4, 5)

bf = ml_dtypes.bfloat16


def _build(repeat=1, mode="full"):
    nc = bacc.Bacc(
        "TRN2", target_bir_lowering=False, debug=False, num_devices=NCORES
    )

    pair_d = nc.dram_tensor("pair_s", [NIJ, C], F32, kind="ExternalInput").ap()
    x_d = nc.dram_tensor("x_s", [N, C], F32, kind="ExternalInput").ap()
    wqkvt_d = nc.dram_tensor("wqkvt", [C, 3 * C], BF16, kind="ExternalInput").ap()
    bqkv_d = nc.dram_tensor("bqkv", [1, 3 * C], BF16, kind="ExternalInput").ap()
    wprojt_d = nc.dram_tensor("wprojt", [C, C], BF16, kind="ExternalInput").ap()
    bproj_d = nc.dram_tensor("bproj", [1, C], BF16, kind="ExternalInput").ap()
    wg_d = nc.dram_tensor("wg13", [C, H + 1], BF16, kind="ExternalInput").ap()
    ncs_d = nc.dram_tensor("ncs", [1, H], BF16, kind="ExternalInput").ap()
    reps_d = nc.dram_tensor("lnreps", [6, C], BF16, kind="ExternalInput").ap()
    out_d = nc.dram_tensor("out", [IB, C], F32, kind="ExternalOutput").ap()

    with tile.TileContext(nc) as tc, ExitStack() as ctx:
        sing = ctx.enter_context(tc.tile_pool(name="sing", bufs=1))
        pairp = ctx.enter_context(tc.tile_pool(name="pairp", bufs=5))
        dumpp = ctx.enter_context(tc.tile_pool(name="dumpp", bufs=2))
        statp = ctx.enter_context(tc.tile_pool(name="statp", bufs=3))
        ptp = ctx.enter_context(tc.tile_pool(name="ptp", bufs=2))
        stagep = ctx.enter_context(tc.tile_pool(name="stagep", bufs=3))
        attnp = ctx.enter_context(tc.tile_pool(name="attnp", bufs=2))
        ps_t = ctx.enter_context(tc.tile_pool(name="ps_t", bufs=5, space="PSUM"))
        ps_mm = ctx.enter_context(tc.tile_pool(name="ps_mm", bufs=2, space="PSUM"))
        ps_s = ctx.enter_context(tc.tile_pool(name="ps_s", bufs=1, space="PSUM"))

        # ---- singles / weights ----
        id128 = sing.tile([128, 128], BF16)
        make_identity(nc, id128)
        ones_col = sing.tile([1, 128], BF16)
        nc.vector.memset(ones_col, 1.0)

        wg = sing.tile([128, 6, H + 1], BF16)
        nc.sync.dma_start(out=wg, in_=wg_d.rearrange("(k p) o -> p k o", p=128))
        # -cs[h] replicated over the 64 i-row partitions
        ncsrep = sing.tile([IB, H], BF16)
        nc.gpsimd.dma_start(
            out=ncsrep,
            in_=bass.AP(tensor=ncs_d.tensor, offset=0, ap=[[0, IB], [1, H]]),
        )

        # deferred-stats accumulator: col = 4*panel + jblock
        s2b = sing.tile([128, 2, 128], BF16)  # sum(pair^2) per row
        # bias panels land here directly, [i, h(12)+mu768(1), j]
        bias_r = sing.tile([IB, H + 1, N], BF16)
        rmap = sing.tile([IB, N], BF16)
        mu_map = sing.tile([IB, N], BF16)

        # ================= compute body =================
        from contextlib import nullcontext

        loop_cm = tc.For_i(0, repeat, 1) if repeat > 1 else nullcontext()
        with loop_cm:
            _compute(nc, tc, locals(), mode)

    nc.compile()
    return nc


def _x_path_a(nc, g):
    """x-path stage A: weight DMAs, LN(x), xn apply. No PE work."""
    sing, statp = g.sing, g.statp
    x_d = g.x_d

    wqkvt = sing.tile([128, 6, 3 * C], BF16)
    nc.sync.dma_start(out=wqkvt, in_=g.wqkvt_d.rearrange("(k p) o -> p k o", p=128))
    bqkv = sing.tile([1, 3 * C], BF16)
    nc.sync.dma_start(out=bqkv, in_=g.bqkv_d)
    # replicated LN params: rows = ln_g, ln_b, qln_g, qln_b, kln_g, kln_b
    reps = sing.tile([128, 6, C], BF16)
    for rI in range(6):
        nc.gpsimd.dma_start(
            out=reps[:, rI, :],
            in_=bass.AP(
                tensor=g.reps_d.tensor, offset=rI * C, ap=[[0, 128], [1, C]]
            ),
        )

    x_sb = sing.tile([128, 4, C], F32)
    nc.sync.dma_start(out=x_sb, in_=x_d.rearrange("(t p) c -> p t c", p=128))
    xn = sing.tile([128, 4, C], BF16)

    bnx = statp.tile([128, 8, 6], F32, tag="bnx")
    mvx = statp.tile([128, 4, 2], F32, tag="mvx")
    rxx = statp.tile([128, 4], F32, tag="rx")
    xv = x_sb.rearrange("p t (k c) -> p t k c", k=2)
    for t in range(4):
        for k in range(2):
            nc.vector.bn_stats(out=bnx[:, 2 * t + k, :], in_=xv[:, t, k, :])
    for t in range(4):
        nc.vector.bn_aggr(out=mvx[:, t, :], in_=bnx[:, 2 * t : 2 * t + 2, :])
    nc.vector.tensor_scalar(
        out=rxx, in0=mvx[:, :, 1], scalar1=EPS, scalar2=None, op0=OP.add
    )
    nc.vector.reciprocal(out=rxx, in_=rxx)
    nc.scalar.activation(out=rxx, in_=rxx, func=AF.Sqrt)
    for t in range(4):
        nc.vector.tensor_scalar(
            out=xn[:, t, :], in0=x_sb[:, t, :],
            scalar1=mvx[:, t, 0:1], scalar2=rxx[:, t : t + 1],
            op0=OP.subtract, op1=OP.mult,
        )
        nc.vector.tensor_tensor(
            out=xn[:, t, :], in0=xn[:, t, :], in1=reps[:, 0, :], op=OP.mult
        )
        nc.vector.tensor_tensor(
            out=xn[:, t, :], in0=xn[:, t, :], in1=reps[:, 1, :], op=OP.add
        )
    return wqkvt, bqkv, reps, xn


def _x_path_b(nc, g, wqkvt, bqkv, reps, xn):
    """x-path stage B: transposes, qkv matmul, q/k LN, kT/qT."""
    sing, statp, ps_t, ps_mm = g.sing, g.statp, g.ps_t, g.ps_mm
    id128, ones_col = g.id128, g.ones_col

    # xnT [c, n]
    xnT = sing.tile([128, 6, N], BF16)
    for ch in range(6):
        pst = ps_t.tile([128, N], BF16, tag="pst")
        for t in range(4):
            nc.tensor.transpose(
                pst[:, t * 128 : (t + 1) * 128],
                xn[:, t, ch * 128 : (ch + 1) * 128],
                id128,
            )
        nc.scalar.copy(out=xnT[:, ch, :], in_=pst)

    # qkv natural [n, 3C]
    qkv = sing.tile([128, 4, 3 * C], BF16)
    OCH = [(0, 512), (512, 512), (1024, 512), (1536, 512), (2048, 256)]
    for t in range(4):
        for occ, ocs in OCH:
            pmm = ps_mm.tile([128, N], F32, tag="mm")
            for ch in range(6):
                nc.tensor.matmul(
                    pmm[:, 0:ocs],
                    lhsT=xnT[:, ch, t * 128 : (t + 1) * 128],
                    rhs=wqkvt[:, ch, occ : occ + ocs],
                    start=(ch == 0), stop=False,
                )
            nc.tensor.matmul(
                pmm[:, 0:ocs], lhsT=ones_col[:, 0:128],
                rhs=bqkv[:, occ : occ + ocs], start=False, stop=True,
            )
            if (occ // 512) % 2 == 0:
                nc.vector.tensor_copy(out=qkv[:, t, occ : occ + ocs], in_=pmm[:, 0:ocs])
            else:
                nc.scalar.copy(out=qkv[:, t, occ : occ + ocs], in_=pmm[:, 0:ocs])

    # q/k LN (in place on qkv)
    bnq = statp.tile([128, 16, 6], F32, tag="bnq")
    mvq = statp.tile([128, 8, 2], F32, tag="mvq")
    rqq = statp.tile([128, 8], F32, tag="rq")
    qv = qkv[:, :, 0 : 2 * C].rearrange("p t (k c) -> p t k c", k=4)
    for t in range(4):
        for k in range(4):
            nc.vector.bn_stats(out=bnq[:, 4 * t + k, :], in_=qv[:, t, k, :])
    for col in range(8):
        nc.vector.bn_aggr(out=mvq[:, col, :], in_=bnq[:, 2 * col : 2 * col + 2, :])
    nc.vector.tensor_scalar(
        out=rqq, in0=mvq[:, :, 1], scalar1=EPS, scalar2=None, op0=OP.add
    )
    nc.vector.reciprocal(out=rqq, in_=rqq)
    nc.scalar.activation(out=rqq, in_=rqq, func=AF.Sqrt)
    for t in range(4):
        for qi, off in enumerate((0, C)):
            col = t * 2 + qi
            gr = 2 + 2 * qi  # qln_g row 2, kln_g row 4
            nc.vector.tensor_scalar(
                out=qkv[:, t, off : off + C], in0=qkv[:, t, off : off + C],
                scalar1=mvq[:, col, 0:1], scalar2=rqq[:, col : col + 1],
                op0=OP.subtract, op1=OP.mult,
            )
            nc.vector.tensor_tensor(
                out=qkv[:, t, off : off + C], in0=qkv[:, t, off : off + C],
                in1=reps[:, gr, :], op=OP.mult,
            )
            nc.vector.tensor_tensor(
                out=qkv[:, t, off : off + C], in0=qkv[:, t, off : off + C],
                in1=reps[:, gr + 1, :], op=OP.add,
            )

    # kT [c, n] for all n; qT [c, i] for own rows (0:64 after roll)
    kT = sing.tile([128, 6, N], BF16)
    for ch in range(6):
        pst = ps_t.tile([128, N], BF16, tag="pst")
        for t in range(4):
            nc.tensor.transpose(
                pst[:, t * 128 : (t + 1) * 128],
                qkv[:, t, C + ch * 128 : C + (ch + 1) * 128],
                id128,
            )
        nc.scalar.copy(out=kT[:, ch, :], in_=pst)
    qT = sing.tile([128, 6, IB], BF16)
    pst = ps_t.tile([128, N], BF16, tag="pst")
    for ch in range(6):
        nc.tensor.transpose(
            pst[:, ch * IB : (ch + 1) * IB],
            qkv[0:IB, 0, ch * 128 : (ch + 1) * 128],
            id128[0:IB, 0:IB],
        )
    nc.vector.tensor_copy(out=qT.rearrange("p a b -> p (a b)"), in_=pst[:, 0 : 6 * IB])
    return qkv, xnT, kT, qT


def _compute(nc, tc, env, mode="full"):
    for _k, _v in env.items():
        globals()["_E_" + _k] = _v

    class _G:
        def __getattr__(self, k):
            return globals()["_E_" + k]

    g = _G()
    (sing, pairp, dumpp, statp, ptp, stagep, attnp, ps_t, ps_mm, ps_s) = (
        g.sing, g.pairp, g.dumpp, g.statp, g.ptp, g.stagep, g.attnp,
        g.ps_t, g.ps_mm, g.ps_s,
    )
    id128, ones_col, rmap, mu_map, bias_r, s2b = (
        g.id128, g.ones_col, g.rmap, g.mu_map, g.bias_r, g.s2b
    )
    wg = g.wg
    pair_d, out_d = g.pair_d, g.out_d

    s2f = s2b.rearrange("p a b -> p (a b)")

    if True:
        # ================= pair path =================
        # per panel: 4 tiles [128, C] = one i-row of 512 j
        pv = pair_d.rearrange("(n t p) c -> n p t c", t=4, p=128)
        NP = NIJ // (4 * 128)  # 64 panels
        qkv = xnT = kT = qT = None
        xa = None
        pending = None  # (pT, pnl) whose matmuls are deferred one panel

        def flush_mm():
            nonlocal pending
            if pending is None:
                return
            pT, ppnl = pending
            pending = None
            # bias matmul: psum[0:12] = pair@wg, psum[12] = 768*mu
            bps = ps_mm.tile([128, N], F32, tag="mm")
            for ch in range(6):
                nc.tensor.matmul(
                    bps[0 : H + 1, :], lhsT=wg[:, ch, :], rhs=pT[:, ch, :],
                    start=(ch == 0), stop=(ch == 5),
                )
            stg = stagep.tile([H + 1, N], BF16)
            if ppnl % 2 == 0:
                nc.vector.tensor_copy(out=stg, in_=bps[0 : H + 1, :])
            else:
                nc.scalar.copy(out=stg, in_=bps[0 : H + 1, :])
            nc.sync.dma_start(out=bias_r[ppnl : ppnl + 1, :, :], in_=stg)

        for pnl in range(NP):
            if pnl == 10:
                xa = _x_path_a(nc, g)
            if pnl == 18:
                flush_mm()
                qkv, xnT, kT, qT = _x_path_b(nc, g, *xa)
            grp = pairp.tile([128, 4, C], BF16, tag="grp")
            if mode != "nodma":
                nc.gpsimd.dma_start(out=grp, in_=pv[pnl])
            if mode == "dmaonly":
                keep = statp.tile([128, 4], F32, tag="s2")
                nc.vector.tensor_copy(out=keep, in_=grp[:, 0, 0:4])
                continue
            # --- sumsq passes, split across engines ---
            s2 = statp.tile([128, 4], F32, tag="s2")
            for t in SQ_ACT:
                dmp = dumpp.tile([128, C], BF16, tag="dA")
                nc.scalar.activation(
                    out=dmp, in_=grp[:, t, :], func=AF.Square,
                    accum_out=s2[:, t : t + 1],
                )
            for t in SQ_DVE:
                dmp = dumpp.tile([128, C], BF16, tag="dD")
                nc.vector.scalar_tensor_tensor(
                    out=dmp, in0=grp[:, t, :], scalar=1.0, in1=grp[:, t, :],
                    op0=OP.mult, op1=OP.mult, accum_out=s2[:, t : t + 1],
                )
            # collect sumsq (bf16) into the deferred tile, col = 4*pnl+tt
            nc.gpsimd.tensor_copy(
                out=s2f[:, 4 * pnl : 4 * pnl + 4], in_=s2
            )

            # raw bf16 transpose of 4 tiles -> pT [c=128, 6, 512];
            # this panel's matmuls are emitted during the NEXT panel so the
            # psum->sbuf drains never stall the PE queue
            pT = ptp.tile([128, 6, N], BF16, tag="pT")
            for ch in range(6):
                pst2 = ps_t.tile([128, N], BF16, tag="pst")
                for tt in range(4):
                    nc.tensor.transpose(
                        pst2[:, tt * 128 : (tt + 1) * 128],
                        grp[:, tt, ch * 128 : (ch + 1) * 128],
                        id128,
                    )
                if ch in DR_DVE:
                    nc.vector.tensor_copy(out=pT[:, ch, :], in_=pst2)
                else:
                    nc.scalar.copy(out=pT[:, ch, :], in_=pst2)
            flush_mm()
            pending = (pT, pnl)
        flush_mm()

        # ---- tail: build mu_map, rmap from deferred stats ----
        # mu_map[i, j] = bias_r[i, 12, j] / 768
        nc.vector.tensor_scalar(
            out=mu_map, in0=bias_r[:, H, :], scalar1=RC, scalar2=None, op0=OP.mult
        )
        # s2 transpose: [128, 2, 128] -> s2map [64, 512] (partition blocks of 4)
        s2T = sing.tile([128, 2, 128], BF16)
        for b in range(2):
            rps = ps_s.tile([128, 128], BF16, tag="murT")
            nc.tensor.transpose(rps, s2b[:, b, :], id128)
            nc.scalar.copy(out=s2T[:, b, :], in_=rps)
        s2map = sing.tile([IB, N], BF16)
        for b in range(2):
            nc.sync.dma_start(
                out=s2map[b * 32 : (b + 1) * 32, :], in_=s2T[:, b, :]
            )
        # var = s2/768 - mu^2 ; rmap = sqrt(1/(var+eps))
        va = sing.tile([IB, N], F32)
        nc.gpsimd.tensor_tensor(out=va, in0=mu_map, in1=mu_map, op=OP.mult)
        nc.vector.scalar_tensor_tensor(
            out=va, in0=s2map, scalar=RC, in1=va, op0=OP.mult, op1=OP.subtract
        )
        nc.vector.tensor_scalar(
            out=va, in0=va, scalar1=EPS, scalar2=None, op0=OP.add
        )
        nc.vector.reciprocal(out=va, in_=va)
        nc.scalar.activation(out=rmap, in_=va, func=AF.Sqrt)

        # ================= attention =================
        ncsrep = g.ncsrep
        wprojt = sing.tile([128, 6, C], BF16)
        nc.sync.dma_start(
            out=wprojt, in_=g.wprojt_d.rearrange("(k p) o -> p k o", p=128)
        )
        bproj = sing.tile([1, C], BF16)
        nc.sync.dma_start(out=bproj, in_=g.bproj_d)
        o_sb = sing.tile([IB, H, HD], BF16)
        for h in range(H):
            sps = ps_mm.tile([128, N], F32, tag="mm")
            bp = (h % 2) * 64
            nc.tensor.matmul(
                sps[0:IB, :],
                lhsT=qT[bp : bp + 64, h // 2, :],
                rhs=kT[bp : bp + 64, h // 2, :],
                start=True, stop=True,
            )
            # bfix = (raw - cs_h * mu) * r  (sb dropped: softmax-invariant)
            t1 = attnp.tile([IB, N], BF16, tag="t1")
            nc.vector.scalar_tensor_tensor(
                out=t1, in0=mu_map, scalar=ncsrep[:, h : h + 1],
                in1=bias_r[:, h, :], op0=OP.mult, op1=OP.add,
            )
            bfix = attnp.tile([IB, N], BF16, tag="bfix")
            nc.gpsimd.tensor_tensor(out=bfix, in0=t1, in1=rmap, op=OP.mult)
            sim = attnp.tile([IB, N], F32, tag="sim")
            nc.vector.scalar_tensor_tensor(
                out=sim, in0=sps[0:IB, :], scalar=0.125, in1=bfix,
                op0=OP.mult, op1=OP.add,
            )
            # logits are small (LN'd q/k, tiny weights): exp without max-sub
            esim = attnp.tile([IB, N], F32, tag="esim")
            den = attnp.tile([IB, 1], F32, tag="den")
            nc.scalar.activation(
                out=esim, in_=sim, func=AF.Exp, accum_out=den
            )
            nc.vector.reciprocal(out=den, in_=den)
            attn = attnp.tile([IB, N], BF16, tag="attn")
            nc.vector.tensor_scalar(
                out=attn, in0=esim, scalar1=den, scalar2=None, op0=OP.mult
            )
            aps = ps_t.tile([128, N], BF16, tag="pst")
            for jc in range(4):
                nc.tensor.transpose(
                    aps[:, jc * IB : (jc + 1) * IB],
                    attn[:, jc * 128 : (jc + 1) * 128],
                    id128[0:IB, 0:IB],
                )
            aT = attnp.tile([128, 4, IB], BF16, tag="aT")
            nc.vector.tensor_copy(
                out=aT.rearrange("p a b -> p (a b)"), in_=aps[:, 0 : 4 * IB]
            )
            ops = ps_mm.tile([128, N], F32, tag="mm")
            for jc in range(4):
                nc.tensor.matmul(
                    ops[0:IB, 0:HD],
                    lhsT=aT[:, jc, :],
                    rhs=qkv[:, jc, 2 * C + h * HD : 2 * C + (h + 1) * HD],
                    start=(jc == 0), stop=(jc == 3),
                )
            nc.vector.tensor_copy(out=o_sb[:, h, :], in_=ops[0:IB, 0:HD])

        # ================= output proj =================
        o_fl = o_sb.rearrange("p a b -> p (a b)")
        oT = sing.tile([128, 6, IB], BF16)
        pso = ps_t.tile([128, N], BF16, tag="pst")
        for ch in range(6):
            nc.tensor.transpose(
                pso[:, ch * IB : (ch + 1) * IB],
                o_fl[:, ch * 128 : (ch + 1) * 128],
                id128[0:IB, 0:IB],
            )
        nc.vector.tensor_copy(out=oT.rearrange("p a b -> p (a b)"), in_=pso[:, 0 : 6 * IB])
        out_sb = sing.tile([IB, C], F32)
        for occ, ocs in [(0, 512), (512, 256)]:
            pps = ps_mm.tile([128, N], F32, tag="mm")
            for ch in range(6):
                nc.tensor.matmul(
                    pps[0:IB, 0:ocs],
                    lhsT=oT[:, ch, :],
                    rhs=wprojt[:, ch, occ : occ + ocs],
                    start=(ch == 0), stop=False,
                )
            nc.tensor.matmul(
                pps[0:IB, 0:ocs], lhsT=ones_col[:, 0:IB],
                rhs=bproj[:, occ : occ + ocs], start=False, stop=True,
            )
            nc.vector.tensor_copy(out=out_sb[:, occ : occ + ocs], in_=pps[0:IB, 0:ocs])
        nc.sync.dma_start(out=out_d, in_=out_sb)


_NC = None
_LAST_MAPS = None


def prep_maps(x, pair, ln_g, ln_b, w_qkv, b_qkv, w_proj, b_proj, w_bias,
              pn_g, pn_b, qln_g, qln_b, kln_g, kln_b):
    x = np.asarray(x, np.float32)
    pair = np.asarray(pair, np.float32)
    wqkvt = np.ascontiguousarray(np.asarray(w_qkv, np.float32).T).astype(bf)
    wprojt = np.ascontiguousarray(np.asarray(w_proj, np.float32).T).astype(bf)
    wg_host = np.ascontiguousarray(
        (np.asarray(pn_g, np.float32)[:, None] * np.asarray(w_bias, np.float32).T)
    )
    cs = wg_host.sum(axis=0)  # colsum over c, [H]
    wg13 = np.concatenate(
        [wg_host, np.ones((C, 1), np.float32)], axis=1
    ).astype(bf)  # [C, 13]; col 12 harvests 768*mu
    ncs = (-cs)[None].astype(bf)  # [1, H]
    reps = np.stack(
        [np.asarray(a, np.float32) for a in (ln_g, ln_b, qln_g, qln_b, kln_g, kln_b)]
    ).astype(bf)
    bqkv = np.asarray(b_qkv, np.float32)[None].astype(bf)
    bproj = np.asarray(b_proj, np.float32)[None].astype(bf)

    in_maps = []
    for k in range(NCORES):
        ps = pair[0, k * IB : (k + 1) * IB]  # [64, 512, 768]
        ps = np.roll(ps, -k * IB, axis=1)  # roll j to match rolled x
        xk = np.roll(x[0], -k * IB, axis=0)
        in_maps.append(
            {
                "pair_s": np.ascontiguousarray(ps.reshape(NIJ, C), np.float32),
                "x_s": np.ascontiguousarray(xk, np.float32),
                "wqkvt": wqkvt,
                "bqkv": bqkv,
                "wprojt": wprojt,
                "bproj": bproj,
                "wg13": wg13,
                "ncs": ncs,
                "lnreps": reps,
            }
        )

    return in_maps


def kernel(**inputs):
    global _NC, _LAST_MAPS
    if _NC is None:
        _NC = _build()
    in_maps = prep_maps(**inputs)
    _LAST_MAPS = in_maps
    res = run_bass_kernel_spmd(_NC, in_maps, list(range(NCORES)))
    outs = [res.results[k]["out"] for k in range(NCORES)]
    return np.concatenate(outs, axis=0)[None].astype(np.float32)


# revision 21
# speedup vs baseline: 1.1364x; 1.1364x over previous
"""Trainium2 Bass kernel for nn_Attention_48687749268214.

Self-attention with pair-bias: LN(x) -> qkv -> q/k LN -> heads,
bias = einsum('bijc,hc->bhij', LN(pair), w_bias), softmax(qk/8+bias) @ v -> proj.

Sharding: sequence-shard the i axis across 8 cores (64 query rows each).
Each core gets its pair slice pair[i0:i0+64] (j-rolled so that the core's own
query rows sit at local rows 0:63 in its rolled copy of x), computes its own
64 output rows with no collectives; host concatenates.

Pair-path math (v4): with pn_g folded into wg on host,
  bias[h, ij] = r_ij * (raw[h, ij] - mu_ij * cs[h]) + sb[h]
where raw = pair @ wg, cs = colsum(wg), and sb is constant along j so softmax
drops it (exact). The kernel never normalizes pair and computes NO stats on
the critical path:
  - mu: a 13th all-ones column in wg makes the bias matmul emit 768*mu as
    psum row 12, which rides into bias_r with the raw panels.
  - var: per-tile sum(pair^2) passes split across ACT/DVE/GpSimd engines,
    collected into a [128, 256] tile (col = 4*panel + jblock).
  - tail: cheap [64, 512] map ops build mu_map/rmap, and per-head
    bfix = (raw - cs_h*mu_map) * rmap feeds softmax.
So PE's pair work (transposes + matmuls) depends only on the pair DMA.
"""

import sys

sys.path.insert(0, "/opt/trn_rl_repo")

from contextlib import ExitStack

import ml_dtypes
import numpy as np

import concourse.bass as bass
import concourse.tile as tile
from concourse import bacc, mybir
from concourse.bass_utils import run_bass_kernel_spmd
from concourse.masks import make_identity

F32 = mybir.dt.float32
BF16 = mybir.dt.bfloat16
AF = mybir.ActivationFunctionType
OP = mybir.AluOpType

C = 768
H = 12
HD = 64
N = 512
NCORES = 8
IB = N // NCORES  # 64 i rows per core
NIJ = IB * N  # 32768 pair rows per core
EPS = 1e-5
RC = 1.0 / C

# engine assignment knobs for the per-panel sumsq passes (tile idx 0-3)
SQ_ACT = (0, 1)
SQ_DVE = (2, 3)
# psum->sbuf drain assignment for pair transposes (c-chunk 0-5)
DR_DVE = (1, 2, 4, 5)

bf = ml_dtypes.bfloat16


def _build(repeat=1, mode="full"):
    nc = bacc.Bacc(
        "TRN2", target_bir_lowering=False, debug=False, num_devices=NCORES
    )

    pair_d = nc.dram_tensor("pair_s", [NIJ, C], F32, kind="ExternalInput").ap()
    x_d = nc.dram_tensor("x_s", [N, C], F32, kind="ExternalInput").ap()
    wqkvt_d = nc.dram_tensor("wqkvt", [C, 3 * C], BF16, kind="ExternalInput").ap()
    bqkv_d = nc.dram_tensor("bqkv", [1, 3 * C], BF16, kind="ExternalInput").ap()
    wprojt_d = nc.dram_tensor("wprojt", [C, C], BF16, kind="ExternalInput").ap()
    bproj_d = nc.dram_tensor("bproj", [1, C], BF16, kind="ExternalInput").ap()
    wg_d = nc.dram_tensor("wg13", [C, H + 1], BF16, kind="ExternalInput").ap()
    ncs_d = nc.dram_tensor("ncs", [1, H], BF16, kind="ExternalInput").ap()
    reps_d = nc.dram_tensor("lnreps", [6, C], BF16, kind="ExternalInput").ap()
    out_d = nc.dram_tensor("out", [IB, C], F32, kind="ExternalOutput").ap()

    with tile.TileContext(nc) as tc, ExitStack() as ctx:
        sing = ctx.enter_context(tc.tile_pool(name="sing", bufs=1))
        pairp = ctx.enter_context(tc.tile_pool(name="pairp", bufs=5))
        dumpp = ctx.enter_context(tc.tile_pool(name="dumpp", bufs=2))
        statp = ctx.enter_context(tc.tile_pool(name="statp", bufs=3))
        ptp = ctx.enter_context(tc.tile_pool(name="ptp", bufs=2))
        stagep = ctx.enter_context(tc.tile_pool(name="stagep", bufs=3))
        attnp = ctx.enter_context(tc.tile_pool(name="attnp", bufs=2))
        ps_t = ctx.enter_context(tc.tile_pool(name="ps_t", bufs=4, space="PSUM"))
        ps_mm = ctx.enter_context(tc.tile_pool(name="ps_mm", bufs=2, space="PSUM"))

        # ---- singles / weights ----
        id128 = sing.tile([128, 128], BF16)
        make_identity(nc, id128)
        ones_col = sing.tile([1, 128], BF16)
        nc.vector.memset(ones_col, 1.0)

        wg = sing.tile([128, 6, H + 1], BF16)
        nc.sync.dma_start(out=wg, in_=wg_d.rearrange("(k p) o -> p k o", p=128))
        # -cs[h] replicated over the 64 i-row partitions
        ncsrep = sing.tile([IB, H], BF16)
        nc.gpsimd.dma_start(
            out=ncsrep,
            in_=bass.AP(tensor=ncs_d.tensor, offset=0, ap=[[0, IB], [1, H]]),
        )

        # deferred-stats accumulator: col = 4*panel + jblock
        s2b = sing.tile([128, 2, 128], BF16)  # sum(pair^2) per row
        # bias panels land here directly, [i, h(12)+mu768(1), j]
        bias_r = sing.tile([IB, H + 1, N], BF16)
        rmap = sing.tile([IB, N], BF16)
        mu_map = sing.tile([IB, N], BF16)

        # ================= compute body =================
        from contextlib import nullcontext

        loop_cm = tc.For_i(0, repeat, 1) if repeat > 1 else nullcontext()
        with loop_cm:
            _compute(nc, tc, locals(), mode)

    nc.compile()
    return nc


def _x_path_a(nc, g):
    """x-path stage A: weight DMAs, LN(x), xn apply. No PE work."""
    sing, statp = g.sing, g.statp
    x_d = g.x_d

    wqkvt = sing.tile([128, 6, 3 * C], BF16)
    nc.sync.dma_start(out=wqkvt, in_=g.wqkvt_d.rearrange("(k p) o -> p k o", p=128))
    bqkv = sing.tile([1, 3 * C], BF16)
    nc.sync.dma_start(out=bqkv, in_=g.bqkv_d)
    # replicated LN params: rows = ln_g, ln_b, qln_g, qln_b, kln_g, kln_b
    reps = sing.tile([128, 6, C], BF16)
    for rI in range(6):
        nc.gpsimd.dma_start(
            out=reps[:, rI, :],
            in_=bass.AP(
                tensor=g.reps_d.tensor, offset=rI * C, ap=[[0, 128], [1, C]]
            ),
        )

    x_sb = sing.tile([128, 4, C], F32)
    nc.sync.dma_start(out=x_sb, in_=x_d.rearrange("(t p) c -> p t c", p=128))
    xn = sing.tile([128, 4, C], BF16)

    bnx = statp.tile([128, 8, 6], F32, tag="bnx")
    mvx = statp.tile([128, 4, 2], F32, tag="mvx")
    rxx = statp.tile([128, 4], F32, tag="rx")
    xv = x_sb.rearrange("p t (k c) -> p t k c", k=2)
    for t in range(4):
        for k in range(2):
            nc.vector.bn_stats(out=bnx[:, 2 * t + k, :], in_=xv[:, t, k, :])
    for t in range(4):
        nc.vector.bn_aggr(out=mvx[:, t, :], in_=bnx[:, 2 * t : 2 * t + 2, :])
    nc.vector.tensor_scalar(
        out=rxx, in0=mvx[:, :, 1], scalar1=EPS, scalar2=None, op0=OP.add
    )
    nc.vector.reciprocal(out=rxx, in_=rxx)
    nc.scalar.activation(out=rxx, in_=rxx, func=AF.Sqrt)
    for t in range(4):
        nc.vector.tensor_scalar(
            out=xn[:, t, :], in0=x_sb[:, t, :],
            scalar1=mvx[:, t, 0:1], scalar2=rxx[:, t : t + 1],
            op0=OP.subtract, op1=OP.mult,
        )
        nc.vector.tensor_tensor(
            out=xn[:, t, :], in0=xn[:, t, :], in1=reps[:, 0, :], op=OP.mult
        )
        nc.vector.tensor_tensor(
            out=xn[:, t, :], in0=xn[:, t, :], in1=reps[:, 1, :], op=OP.add
        )
    return wqkvt, bqkv, reps, xn


def _x_path_b(nc, g, wqkvt, bqkv, reps, xn):
    """x-path stage B: transposes, qkv matmul, q/k LN, kT/qT."""
    sing, statp, ps_t, ps_mm = g.sing, g.statp, g.ps_t, g.ps_mm
    id128, ones_col = g.id128, g.ones_col

    # xnT [c, n]
    xnT = sing.tile([128, 6, N], BF16)
    for ch in range(6):
        pst = ps_t.tile([128, N], BF16, tag="pst")
        for t in range(4):
            nc.tensor.transpose(
                pst[:, t * 128 : (t + 1) * 128],
                xn[:, t, ch * 128 : (ch + 1) * 128],
                id128,
            )
        nc.scalar.copy(out=xnT[:, ch, :], in_=pst)

    # qkv natural [n, 3C]
    qkv = sing.tile([128, 4, 3 * C], BF16)
    OCH = [(0, 512), (512, 512), (1024, 512), (1536, 512), (2048, 256)]
    for t in range(4):
        for occ, ocs in OCH:
            pmm = ps_mm.tile([128, N], F32, tag="mm")
            for ch in range(6):
                nc.tensor.matmul(
                    pmm[:, 0:ocs],
                    lhsT=xnT[:, ch, t * 128 : (t + 1) * 128],
                    rhs=wqkvt[:, ch, occ : occ + ocs],
                    start=(ch == 0), stop=False,
                )
            nc.tensor.matmul(
                pmm[:, 0:ocs], lhsT=ones_col[:, 0:128],
                rhs=bqkv[:, occ : occ + ocs], start=False, stop=True,
            )
            if (occ // 512) % 2 == 0:
                nc.vector.tensor_copy(out=qkv[:, t, occ : occ + ocs], in_=pmm[:, 0:ocs])
            else:
                nc.scalar.copy(out=qkv[:, t, occ : occ + ocs], in_=pmm[:, 0:ocs])

    # q/k LN (in place on qkv)
    bnq = statp.tile([128, 16, 6], F32, tag="bnq")
    mvq = statp.tile([128, 8, 2], F32, tag="mvq")
    rqq = statp.tile([128, 8], F32, tag="rq")
    qv = qkv[:, :, 0 : 2 * C].rearrange("p t (k c) -> p t k c", k=4)
    for t in range(4):
        for k in range(4):
            nc.vector.bn_stats(out=bnq[:, 4 * t + k, :], in_=qv[:, t, k, :])
    for col in range(8):
        nc.vector.bn_aggr(out=mvq[:, col, :], in_=bnq[:, 2 * col : 2 * col + 2, :])
    nc.vector.tensor_scalar(
        out=rqq, in0=mvq[:, :, 1], scalar1=EPS, scalar2=None, op0=OP.add
    )
    nc.vector.reciprocal(out=rqq, in_=rqq)
    nc.scalar.activation(out=rqq, in_=rqq, func=AF.Sqrt)
    for t in range(4):
        for qi, off in enumerate((0, C)):
            col = t * 2 + qi
            gr = 2 + 2 * qi  # qln_g row 2, kln_g row 4
            nc.vector.tensor_scalar(
                out=qkv[:, t, off : off + C], in0=qkv[:, t, off : off + C],
                scalar1=mvq[:, col, 0:1], scalar2=rqq[:, col : col + 1],
                op0=OP.subtract, op1=OP.mult,
            )
            nc.vector.tensor_tensor(
                out=qkv[:, t, off : off + C], in0=qkv[:, t, off : off + C],
                in1=reps[:, gr, :], op=OP.mult,
            )
            nc.vector.tensor_tensor(
                out=qkv[:, t, off : off + C], in0=qkv[:, t, off : off + C],
                in1=reps[:, gr + 1, :], op=OP.add,
            )

    # kT [c, n] for all n; qT [c, i] for own rows (0:64 after roll)
    kT = sing.tile([128, 6, N], BF16)
    for ch in range(6):
        pst = ps_t.tile([128, N], BF16, tag="pst")
        for t in range(4):
            nc.tensor.transpose(
                pst[:, t * 128 : (t + 1) * 128],
                qkv[:, t, C + ch * 128 : C + (ch + 1) * 128],
                id128,
            )
        nc.scalar.copy(out=kT[:, ch, :], in_=pst)
    qT = sing.tile([128, 6, IB], BF16)
    pst = ps_t.tile([128, N], BF16, tag="pst")
    for ch in range(6):
        nc.tensor.transpose(
            pst[:, ch * IB : (ch + 1) * IB],
            qkv[0:IB, 0, ch * 128 : (ch + 1) * 128],
            id128[0:IB, 0:IB],
        )
    nc.vector.tensor_copy(out=qT.rearrange("p a b -> p (a b)"), in_=pst[:, 0 : 6 * IB])
    return qkv, xnT, kT, qT


def _compute(nc, tc, env, mode="full"):
    for _k, _v in env.items():
        globals()["_E_" + _k] = _v

    class _G:
        def __getattr__(self, k):
            return globals()["_E_" + k]

    g = _G()
    (sing, pairp, dumpp, statp, ptp, stagep, attnp, ps_t, ps_mm) = (
        g.sing, g.pairp, g.dumpp, g.statp, g.ptp, g.stagep, g.attnp,
        g.ps_t, g.ps_mm,
    )
    id128, ones_col, rmap, mu_map, bias_r, s2b = (
        g.id128, g.ones_col, g.rmap, g.mu_map, g.bias_r, g.s2b
    )
    wg = g.wg
    pair_d, out_d = g.pair_d, g.out_d

    s2f = s2b.rearrange("p a b -> p (a b)")

    if True:
        # ================= pair path =================
        # per panel: 4 tiles [128, C] = one i-row of 512 j
        pv = pair_d.rearrange("(n t p) c -> n p t c", t=4, p=128)
        NP = NIJ // (4 * 128)  # 64 panels
        qkv = xnT = kT = qT = None
        xa = None
        pending = None  # (pT, pnl) whose matmuls are deferred one panel

        s2map = sing.tile([IB, N], BF16)
        va = sing.tile([IB, N], F32)

        def emit_maps_half(b):
            # s2 transpose block b -> s2map rows [32b:32b+32]; then
            # mu_map/rmap for those i rows (panels 32b..32b+31 complete)
            lo = 32 * b
            rps = ps_t.tile([128, N], BF16, tag="pst")
            nc.tensor.transpose(rps[:, 0:128], s2b[:, b, :], id128)
            s2T = statp.tile([128, 128], BF16, tag="s2T")
            nc.scalar.copy(out=s2T, in_=rps[:, 0:128])
            nc.sync.dma_start(out=s2map[lo : lo + 32, :], in_=s2T)
            nc.vector.tensor_scalar(
                out=mu_map[lo : lo + 32, :], in0=bias_r[lo : lo + 32, H, :],
                scalar1=RC, scalar2=None, op0=OP.mult,
            )
            nc.gpsimd.tensor_tensor(
                out=va[lo : lo + 32, :], in0=mu_map[lo : lo + 32, :],
                in1=mu_map[lo : lo + 32, :], op=OP.mult,
            )
            nc.vector.scalar_tensor_tensor(
                out=va[lo : lo + 32, :], in0=s2map[lo : lo + 32, :], scalar=RC,
                in1=va[lo : lo + 32, :], op0=OP.mult, op1=OP.subtract,
            )
            nc.vector.tensor_scalar(
                out=va[lo : lo + 32, :], in0=va[lo : lo + 32, :],
                scalar1=EPS, scalar2=None, op0=OP.add,
            )
            nc.vector.reciprocal(out=va[lo : lo + 32, :], in_=va[lo : lo + 32, :])
            nc.scalar.activation(
                out=rmap[lo : lo + 32, :], in_=va[lo : lo + 32, :], func=AF.Sqrt
            )

        def flush_mm():
            nonlocal pending
            if pending is None:
                return
            pT, ppnl = pending
            pending = None
            # bias matmul: psum[0:12] = pair@wg, psum[12] = 768*mu
            bps = ps_mm.tile([128, N], F32, tag="mm")
            for ch in range(6):
                nc.tensor.matmul(
                    bps[0 : H + 1, :], lhsT=wg[:, ch, :], rhs=pT[:, ch, :],
                    start=(ch == 0), stop=(ch == 5),
                )
            stg = stagep.tile([H + 1, N], BF16)
            if ppnl % 2 == 0:
                nc.vector.tensor_copy(out=stg, in_=bps[0 : H + 1, :])
            else:
                nc.scalar.copy(out=stg, in_=bps[0 : H + 1, :])
            nc.sync.dma_start(out=bias_r[ppnl : ppnl + 1, :, :], in_=stg)

        for pnl in range(NP):
            if pnl == 10:
                xa = _x_path_a(nc, g)
            if pnl == 18:
                flush_mm()
                qkv, xnT, kT, qT = _x_path_b(nc, g, *xa)
            grp = pairp.tile([128, 4, C], BF16, tag="grp")
            if mode != "nodma":
                nc.gpsimd.dma_start(out=grp, in_=pv[pnl])
            if mode == "dmaonly":
                keep = statp.tile([128, 4], F32, tag="s2")
                nc.vector.tensor_copy(out=keep, in_=grp[:, 0, 0:4])
                continue
            # --- sumsq passes, split across engines ---
            s2 = statp.tile([128, 4], F32, tag="s2")
            for t in SQ_ACT:
                dmp = dumpp.tile([128, C], BF16, tag="dA")
                nc.scalar.activation(
                    out=dmp, in_=grp[:, t, :], func=AF.Square,
                    accum_out=s2[:, t : t + 1],
                )
            for t in SQ_DVE:
                dmp = dumpp.tile([128, C], BF16, tag="dD")
                nc.vector.scalar_tensor_tensor(
                    out=dmp, in0=grp[:, t, :], scalar=1.0, in1=grp[:, t, :],
                    op0=OP.mult, op1=OP.mult, accum_out=s2[:, t : t + 1],
                )
            # collect sumsq (bf16) into the deferred tile, col = 4*pnl+tt
            nc.gpsimd.tensor_copy(
                out=s2f[:, 4 * pnl : 4 * pnl + 4], in_=s2
            )

            # raw bf16 transpose of 4 tiles -> pT [c=128, 6, 512];
            # this panel's matmuls are emitted during the NEXT panel so the
            # psum->sbuf drains never stall the PE queue
            pT = ptp.tile([128, 6, N], BF16, tag="pT")
            for ch in range(6):
                pst2 = ps_t.tile([128, N], BF16, tag="pst")
                for tt in range(4):
                    nc.tensor.transpose(
                        pst2[:, tt * 128 : (tt + 1) * 128],
                        grp[:, tt, ch * 128 : (ch + 1) * 128],
                        id128,
                    )
                if ch in DR_DVE:
                    nc.vector.tensor_copy(out=pT[:, ch, :], in_=pst2)
                else:
                    nc.scalar.copy(out=pT[:, ch, :], in_=pst2)
            flush_mm()
            pending = (pT, pnl)
            if pnl == 33:
                emit_maps_half(0)
        flush_mm()
        emit_maps_half(1)

        # ================= attention =================
        ncsrep = g.ncsrep
        wprojt = sing.tile([128, 6, C], BF16)
        nc.sync.dma_start(
            out=wprojt, in_=g.wprojt_d.rearrange("(k p) o -> p k o", p=128)
        )
        bproj = sing.tile([1, C], BF16)
        nc.sync.dma_start(out=bproj, in_=g.bproj_d)
        o_sb = sing.tile([IB, H, HD], BF16)
        # bfix[h] = (raw_h - cs_h * mu) * r  (sb dropped: softmax-invariant)
        bfull = sing.tile([IB, H, N], BF16)
        for h in range(H):
            t1 = attnp.tile([IB, N], BF16, tag="t1")
            nc.vector.scalar_tensor_tensor(
                out=t1, in0=mu_map, scalar=ncsrep[:, h : h + 1],
                in1=bias_r[:, h, :], op0=OP.mult, op1=OP.add,
            )
            nc.gpsimd.tensor_tensor(
                out=bfull[:, h, :], in0=t1, in1=rmap, op=OP.mult
            )
        for h in range(H):
            sps = ps_mm.tile([IB, N], F32, tag="sps")
            bp = (h % 2) * 64
            nc.tensor.matmul(
                sps[0:IB, :],
                lhsT=qT[bp : bp + 64, h // 2, :],
                rhs=kT[bp : bp + 64, h // 2, :],
                start=True, stop=True,
            )
            sim = attnp.tile([IB, N], F32, tag="sim")
            nc.vector.scalar_tensor_tensor(
                out=sim, in0=sps[0:IB, :], scalar=0.125, in1=bfull[:, h, :],
                op0=OP.mult, op1=OP.add,
            )
            # logits are small (LN'd q/k, tiny weights): exp without max-sub
            esim = attnp.tile([IB, N], F32, tag="esim")
            den = attnp.tile([IB, 1], F32, tag="den")
            nc.scalar.activation(
                out=esim, in_=sim, func=AF.Exp, accum_out=den
            )
            nc.vector.reciprocal(out=den, in_=den)
            attn = attnp.tile([IB, N], BF16, tag="attn")
            nc.vector.tensor_scalar(
                out=attn, in0=esim, scalar1=den, scalar2=None, op0=OP.mult
            )
            aps = ps_t.tile([128, N], BF16, tag="pst")
            for jc in range(4):
                nc.tensor.transpose(
                    aps[:, jc * IB : (jc + 1) * IB],
                    attn[:, jc * 128 : (jc + 1) * 128],
                    id128[0:IB, 0:IB],
                )
            aT = attnp.tile([128, 4, IB], BF16, tag="aT")
            nc.vector.tensor_copy(
                out=aT.rearrange("p a b -> p (a b)"), in_=aps[:, 0 : 4 * IB]
            )
            ops = ps_mm.tile([128, N], F32, tag="mm")
            for jc in range(4):
                nc.tensor.matmul(
                    ops[0:IB, 0:HD],
                    lhsT=aT[:, jc, :],
                    rhs=qkv[:, jc, 2 * C + h * HD : 2 * C + (h + 1) * HD],
                    start=(jc == 0), stop=(jc == 3),
                )
            nc.vector.tensor_copy(out=o_sb[:, h, :], in_=ops[0:IB, 0:HD])

        # ================= output proj =================
        o_fl = o_sb.rearrange("p a b -> p (a b)")
        oT = sing.tile([128, 6, IB], BF16)
        pso = ps_t.tile([128, N], BF16, tag="pst")
        for ch in range(6):
            nc.tensor.transpose(
                pso[:, ch * IB : (ch + 1) * IB],
                o_fl[:, ch * 128 : (ch + 1) * 128],
                id128[0:IB, 0:IB],
            )
        nc.vector.tensor_copy(out=oT.rearrange("p a b -> p (a b)"), in_=pso[:, 0 : 6 * IB])
        out_sb = sing.tile([IB, C], F32)
        for occ, ocs in [(0, 512), (512, 256)]:
            pps = ps_mm.tile([128, N], F32, tag="mm")
            for ch in range(6):
                nc.tensor.matmul(
                    pps[0:IB, 0:ocs],
                    lhsT=oT[:, ch, :],
                    rhs=wprojt[:, ch, occ : occ + ocs],
                    start=(ch == 0), stop=False,
                )
            nc.tensor.matmul(
                pps[0:IB, 0:ocs], lhsT=ones_col[:, 0:IB],
                rhs=bproj[:, occ : occ + ocs], start=False, stop=True,
            )
            nc.vector.tensor_copy(out=out_sb[:, occ : occ + ocs], in_=pps[0:IB, 0:ocs])
        nc.sync.dma_start(out=out_d, in_=out_sb)


_NC = None
_LAST_MAPS = None


def prep_maps(x, pair, ln_g, ln_b, w_qkv, b_qkv, w_proj, b_proj, w_bias,
              pn_g, pn_b, qln_g, qln_b, kln_g, kln_b):
    x = np.asarray(x, np.float32)
    pair = np.asarray(pair, np.float32)
    wqkvt = np.ascontiguousarray(np.asarray(w_qkv, np.float32).T).astype(bf)
    wprojt = np.ascontiguousarray(np.asarray(w_proj, np.float32).T).astype(bf)
    wg_host = np.ascontiguousarray(
        (np.asarray(pn_g, np.float32)[:, None] * np.asarray(w_bias, np.float32).T)
    )
    cs = wg_host.sum(axis=0)  # colsum over c, [H]
    wg13 = np.concatenate(
        [wg_host, np.ones((C, 1), np.float32)], axis=1
    ).astype(bf)  # [C, 13]; col 12 harvests 768*mu
    ncs = (-cs)[None].astype(bf)  # [1, H]
    reps = np.stack(
        [np.asarray(a, np.float32) for a in (ln_g, ln_b, qln_g, qln_b, kln_g, kln_b)]
    ).astype(bf)
    bqkv = np.asarray(b_qkv, np.float32)[None].astype(bf)
    bproj = np.asarray(b_proj, np.float32)[None].astype(bf)

    in_maps = []
    for k in range(NCORES):
        ps = pair[0, k * IB : (k + 1) * IB]  # [64, 512, 768]
        ps = np.roll(ps, -k * IB, axis=1)  # roll j to match rolled x
        xk = np.roll(x[0], -k * IB, axis=0)
        in_maps.append(
            {
                "pair_s": np.ascontiguousarray(ps.reshape(NIJ, C), np.float32),
                "x_s": np.ascontiguousarray(xk, np.float32),
                "wqkvt": wqkvt,
                "bqkv": bqkv,
                "wprojt": wprojt,
                "bproj": bproj,
                "wg13": wg13,
                "ncs": ncs,
                "lnreps": reps,
            }
        )

    return in_maps


def kernel(**inputs):
    global _NC, _LAST_MAPS
    if _NC is None:
        _NC = _build()
    in_maps = prep_maps(**inputs)
    _LAST_MAPS = in_maps
    res = run_bass_kernel_spmd(_NC, in_maps, list(range(NCORES)))
    outs = [res.results[k]["out"] for k in range(NCORES)]
    return np.concatenate(outs, axis=0)[None].astype(np.float32)
